# revision 13
# baseline (speedup 1.0000x reference)
"""Trainium2 Bass kernel for ContinuousWaveletLayer (CWT energy).

Reference computation:
  bank = Morlet wavelet bank [32 scales, Lmax=256] (static)
  coef[b,s,t] = 'same' conv of x[b,:] (len 8192) with bank[s,:]
  out[b,s]    = mean_t(coef^2) * softmax(scale_weights)[s]

Device strategy (8 NeuronCores, scale-parallel, 4 scales/core):
  Toeplitz matmuls on the tensor engine in fp8 (e4m3).  Each core gets
  2 "small" scales (L=8s <= 128) and 2 "large" ones.  Small scales are
  realigned per scale (their own time-shifted copy of x) so one output
  block needs a 2-block input window -> ONE DoubleRow matmul (K=256,
  2 fp8 weights per PE cell).  Large scales use the centered layout:
  a DoubleRow matmul for the D=0,1 pair plus a normal fp8 matmul for
  D=2, accumulated in PSUM.  Squares evacuate PSUM on ACT (3 of 4
  scales -> fp8 direct) and DVE (bf16 copy + bf16 mul).  Per-scale
  time-reduction is a DoubleRow "selector" matmul reducing TWO scales'
  squares at once into a [2,512] PSUM accumulator (K=256, accumulated
  over all 16 block groups).  The input DMA is split into 13 chunks so
  conv starts ~3us after the DMA begins; dummy warm-up matmuls keep
  the PE HAM un-throttled during the DMA; standalone weight-loads
  "prefence" each DMA/cross-engine dependency so every matmul carries
  at most one sync wait.  Host folds the 4 sub-block columns, applies
  1/N and softmax.
"""

import os
import sys
from contextlib import ExitStack

import numpy as np

sys.path.insert(0, "/opt/trn_rl_repo")

import concourse.bass as bass
import concourse.mybir as mybir
from concourse import tile
from concourse.ap import AP
from concourse.bass_utils import run_bass_kernel_spmd
from concourse.vector_clock import ScopedClock


def _drain_and_barrier_single_wait(self, tick_clock, wait_clock):
    """TileContext._drain_and_barrier, but the kernel-tail drain's
    global-clock waits are spread over a chain of single-wait drains —
    the walrus build here allows only one sync wait per instruction."""
    drain_inst = self.nc.sync.drain()
    wait_clock.add_sem_waits(
        drain_inst.ins, ScopedClock({None: tick_clock.global_clock})
    )
    si = drain_inst.ins.sync_info
    waits = list(si.on_wait)
    if len(waits) > 1:
        si.on_wait = [waits[0]]
        sems = {h.name: h for h in self.sems.allocated().values()}
        for w in waits[1:]:
            d2 = self.nc.sync.drain()
            d2.wait_op(sems[w.ant_name], w.wait_value, "sem-ge")
    self.nc.all_engine_barrier()
    assert self.sems is not None
    popped = self.nc._tile_sem_poison_stack.pop()
    assert popped is self._sem_poison
    self.nc.clear_and_free_semaphores(list(self.sems.allocated().values()))
    self.nc.all_engine_barrier()


tile.TileContext._drain_and_barrier = _drain_and_barrier_single_wait

N_CORES = 8
S_TOTAL = 32          # number of scales
S_PER = 4             # scales per core
P = 128               # partition / block size
NT = 8192             # time samples
LMAX = 256            # padded kernel length
NBLK = 66             # input blocks: (128 + 8192 + 128) / 128
NGRP = 16             # groups of 4 output blocks (N=512 matmuls)
F32 = mybir.dt.float32
BF16 = mybir.dt.bfloat16
FP8 = mybir.dt.float8e4
DR = mybir.MatmulPerfMode.DoubleRow

# slot -> (g col base, x array, n D-blocks); slots 0,1 small; 2,3 large
SLOTCFG = [(0, 0, 2), (256, 1, 2), (512, 2, 3), (896, 2, 3)]
GCOLS = 1280
SELBASE = GCOLS                # DoubleRow reduce selector (32 cols)
XBASE = GCOLS + 32
CHUNK_BLKS = 18                # x chunk k holds blocks [16k, 16k+18)
CHUNK_COLS = CHUNK_BLKS * P    # 2304
NCHUNK = 4
XARR = NCHUNK * CHUNK_COLS     # 9216 cols per x array (xsA, xsB, xL)
COLS = XBASE + 3 * XARR
NWARM = 18                     # PE warm-up matmuls during the input DMA

LAST_RESULTS = None   # BassKernelResults of the most recent run (for test.py)


def _morlet_kernel_bank(n_scales: int, n: int) -> np.ndarray:
    Lmax = min(8 * n_scales, n)
    bank = np.zeros((n_scales, Lmax), dtype=np.float32)
    for i, s in enumerate(range(1, n_scales + 1)):
        L = min(8 * s, n)
        t = np.linspace(-4.0 * s, 4.0 * s, L)
        w = np.exp(-t**2 / (2.0 * s**2)) * np.cos(5.0 * t / s)
        w = w / np.sqrt(s)
        off = (Lmax - 1) // 2 - (L - 1) // 2
        bank[i, off : off + L] = w.astype(np.float32)
    return bank


_A = np.arange(P)[:, None]
_TO = np.arange(P)[None, :]


def _toeplitz(gtaps: np.ndarray, ndblk: int) -> np.ndarray:
    """[ndblk, 128, 128] blocks: G[D][a,to] = gtaps[128*D + a - to]."""
    L = len(gtaps)
    out = np.zeros((ndblk, P, P), dtype=np.float32)
    for D in range(ndblk):
        d = 128 * D + _A - _TO
        valid = (d >= 0) & (d < L)
        out[D] = np.where(valid, gtaps[np.clip(d, 0, L - 1)], 0.0)
    return out


def _core_scales(c: int) -> list[int]:
    """Global 0-based scale indices for core c's 4 slots."""
    return [2 * c, 2 * c + 1, 16 + 2 * c, 17 + 2 * c]


def _strided(sl, dims):
    """Manual AP on a tile slice: dims = [(stride, n), ...] free dims,
    partition dim inherited from the 2D slice."""
    return AP(
        tensor=sl.tensor,
        offset=sl.offset,
        ap=[list(sl.ap[0])] + [[st, n] for st, n in dims],
    )


def _build_nc() -> bass.Bass:
    nc = bass.Bass()
    xg = nc.dram_tensor("xg", [P, COLS], FP8, kind="ExternalInput")
    # per-core partial energies: [2, (pair, Bsub, b)]; slot = 2*pair + row
    outp = nc.dram_tensor("outp", [2, 1024], F32, kind="ExternalOutput")

    with tile.TileContext(nc) as tc, ExitStack() as ctx:
        xpool = ctx.enter_context(tc.tile_pool(name="x", bufs=1))
        wupool = ctx.enter_context(tc.tile_pool(name="wu", bufs=1))
        sqpool = ctx.enter_context(tc.tile_pool(name="sq", bufs=NGRP * 2))
        cppool = ctx.enter_context(tc.tile_pool(name="cp", bufs=4))
        rowpool = ctx.enter_context(tc.tile_pool(name="row", bufs=1))
        pspool = ctx.enter_context(tc.tile_pool(name="ps", bufs=5, space="PSUM"))
        wtpool = ctx.enter_context(tc.tile_pool(name="wt", bufs=1, space="PSUM"))
        psepool = ctx.enter_context(tc.tile_pool(name="pse", bufs=1, space="PSUM"))

        xgsb = xpool.tile([P, COLS], FP8)

        # PE warm-up: keep the HAM un-throttled while the input DMA runs.
        dmy = wupool.tile([P, 256], BF16)
        nc.gpsimd.memset(dmy[:, :], 0.0)
        wt = wtpool.tile([P, 256], F32, tag="wt", name="wt")
        for w in range(NWARM):
            nc.tensor.matmul(
                wt[:, :], dmy[:, :P], dmy[:, :],
                start=(w == 0), stop=(w == NWARM - 1),
            )

        # chunked input DMA: weights+selector first, then the three x
        # arrays chunk-tier by chunk-tier
        nc.sync.dma_start(out=xgsb[:, :XBASE], in_=xg[:, :XBASE])
        for k in range(NCHUNK):
            # one strided DMA moves chunk k of all three x arrays
            c0 = XBASE + k * CHUNK_COLS
            dims = [[XARR, 3], [1, CHUNK_COLS]]
            nc.sync.dma_start(
                out=_strided(xgsb[:, c0 : c0 + CHUNK_COLS], dims),
                in_=_strided(xg[:, c0 : c0 + CHUNK_COLS], dims),
            )

        # prefence the weights DMA
        nc.tensor.ldweights(xgsb[:, :P])

        # per-pair PSUM energy accumulators [2, 512]
        pes = [
            psepool.tile([2, 512], F32, tag=f"pe{pr}", name=f"pe{pr}")
            for pr in range(2)
        ]

        for ng in range(NGRP):
            k = ng // 4
            loc = (ng % 4) * 4 * P
            if ng % 4 == 0:
                # chunk-tier prefence (one DMA covers all three arrays)
                c0 = XBASE + k * CHUNK_COLS
                nc.tensor.ldweights(xgsb[:, c0 : c0 + P])
            # ACT evacuates 3 of 4 PSUM tiles (it is faster per tile);
            # every 4th group DVE takes two to balance totals
            dve_slots = {2, 3} if (ng % 4 == 3 and ng != NGRP - 1) else {3}
            sqtiles = {}
            for sl in range(S_PER):
                gbase, arr, nd = SLOTCFG[sl]
                base = XBASE + arr * XARR + k * CHUNK_COLS + loc
                pt = pspool.tile([P, 512], F32)
                # DoubleRow conv: D=0,1 fused (K=256)
                lhsT_dr = _strided(xgsb[:, gbase : gbase + P], [[P, 2], [1, P]])
                rhs_dr = _strided(xgsb[:, base : base + 512], [[P, 2], [1, 512]])
                nc.tensor.matmul(
                    pt[:, :], lhsT_dr, rhs_dr,
                    start=True, stop=(nd == 2), perf_mode=DR,
                )
                if nd == 3:  # large scale: normal fp8 matmul for D=2
                    nc.tensor.matmul(
                        pt[:, :],
                        xgsb[:, gbase + 2 * P : gbase + 3 * P],
                        xgsb[:, base + 2 * P : base + 2 * P + 512],
                        start=False, stop=True,
                    )
                pr, half = sl // 2, sl % 2
                if half == 0:
                    sqp = sqpool.tile([P, 1024], FP8)
                    sqtiles[pr] = sqp
                else:
                    sqp = sqtiles[pr]
                dst = sqp[:, half * 512 : half * 512 + 512]
                if sl not in dve_slots:
                    # ACT: square+cast straight out of PSUM
                    nc.scalar.square(dst, pt[:, :])
                else:
                    # DVE: bf16 copy out of PSUM, then bf16 square
                    cp = cppool.tile([P, 512], BF16)
                    nc.vector.tensor_copy(cp[:, :], pt[:, :])
                    nc.vector.tensor_mul(dst, cp[:, :], cp[:, :])
                if half == 1:
                    # DoubleRow selector reduce: both scales of the pair
                    lhsT_red = _strided(
                        xgsb[:, SELBASE : SELBASE + 2], [[16, 2], [1, 2]]
                    )
                    rhs_red = _strided(sqp[:, 0:512], [[512, 2], [1, 512]])
                    if pr == 1 and dve_slots == {3}:
                        # fence: absorb the DVE wait so the mixed-writer
                        # pair's reduce carries a single sync wait
                        nc.tensor.ldweights(sqp[:, 512 : 512 + P])
                    nc.tensor.matmul(
                        pes[pr][:, :], lhsT_red, rhs_red,
                        start=(ng == 0), stop=(ng == NGRP - 1), perf_mode=DR,
                    )

        # tail: evict the two [2,512] accumulators in parallel, two DMAs
        rowout = rowpool.tile([2, 1024], F32, tag="rowout", name="rowout")
        nc.scalar.copy(rowout[:, 0:512], pes[0][:, :])
        nc.vector.tensor_copy(rowout[:, 512:1024], pes[1][:, :])
        nc.sync.dma_start(out=outp[:, 0:512], in_=rowout[:, 0:512])
        nc.sync.dma_start(out=outp[:, 512:1024], in_=rowout[:, 512:1024])

    return nc


_NC_CACHE = None


def _get_nc() -> bass.Bass:
    global _NC_CACHE
    if _NC_CACHE is None:
        _NC_CACHE = _build_nc()
    return _NC_CACHE


def _block_chunks(xs2: np.ndarray) -> np.ndarray:
    """[NBLK*P, P] time-major -> [P, XARR] blocked, 4 chunks of 18 blocks."""
    xb2 = np.ascontiguousarray(
        xs2.reshape(NBLK, P, P).transpose(1, 0, 2).reshape(P, NBLK * P)
    )
    return np.concatenate(
        [xb2[:, 16 * k * P : (16 * k + CHUNK_BLKS) * P] for k in range(NCHUNK)],
        axis=1,
    )


def kernel(x: np.ndarray, scale_weights: np.ndarray, _trace: bool = False) -> np.ndarray:
    global LAST_RESULTS
    x = np.asarray(x, dtype=np.float32)
    scale_weights = np.asarray(scale_weights, dtype=np.float32)
    assert x.shape == (P, NT) and scale_weights.shape == (S_TOTAL,)

    import ml_dtypes

    fp8 = ml_dtypes.float8_e4m3fn

    bank = _morlet_kernel_bank(S_TOTAL, NT)     # [32, 256]
    grev = bank[:, ::-1].copy()                 # reversed rows

    # host prep: zero-pad, per-shift time-major blocked layouts
    xpad = np.zeros((NBLK * P, P), dtype=np.float32)
    xpad[P : P + NT, :] = x.T
    xchL = _block_chunks(xpad)

    # DoubleRow reduce selector [128, 32]:
    # cols 0,1 = ko0 weights (slot A -> row 0), cols 16,17 = ko1 (slot B)
    sel = np.zeros((P, 32), dtype=np.float32)
    sel[:, 0] = 1.0
    sel[:, 17] = 1.0

    xgs = []
    for c in range(N_CORES):
        scales = _core_scales(c)
        gw = np.zeros((P, GCOLS), dtype=np.float32)
        xarrs = []
        for sl in range(S_PER):
            gbase, arr, nd = SLOTCFG[sl]
            sidx = scales[sl]
            if nd == 2:  # small scale: realigned taps + shifted x copy
                s = sidx + 1
                off, L = 128 - 4 * s, 8 * s
                gtaps = grev[sidx][off : off + L]
                xs2 = np.zeros_like(xpad)
                xs2[: NBLK * P - off, :] = xpad[off:, :]
                xarrs.append(_block_chunks(xs2))
            else:
                gtaps = grev[sidx]
            G = _toeplitz(np.asarray(gtaps, dtype=np.float64), nd)
            for D in range(nd):
                gw[:, gbase + D * P : gbase + (D + 1) * P] = G[D]
        xarrs.append(xchL)
        xgs.append(
            np.ascontiguousarray(
                np.concatenate([gw, sel] + xarrs, axis=1).astype(fp8)
            )
        )

    nc = _get_nc()
    in_maps = [{"xg": xgs[c]} for c in range(N_CORES)]
    res = run_bass_kernel_spmd(nc, in_maps, list(range(N_CORES)), trace=_trace)
    LAST_RESULTS = res

    # gather + unshard: outp[c] is [2, (pair, Bsub, b)] f32; slot = 2pr+row
    esum = np.zeros((S_TOTAL, P), dtype=np.float64)
    for c in range(N_CORES):
        scales = _core_scales(c)
        arr = np.asarray(res.results[c]["outp"], dtype=np.float64)  # [2, 1024]
        for pr in range(2):
            blk = arr[:, pr * 512 : (pr + 1) * 512].reshape(2, 4, P).sum(axis=1)
            esum[scales[2 * pr + 0]] = blk[0]
            esum[scales[2 * pr + 1]] = blk[1]
    energy = (esum.T / np.float64(NT)).astype(np.float32)

    w = scale_weights.astype(np.float64)
    e = np.exp(w - w.max())
    sm = (e / e.sum()).astype(np.float32)
    return (energy * sm[None, :]).astype(np.float32)


if __name__ == "__main__":
    rng = np.random.default_rng(0)
    x = rng.standard_normal((P, NT), dtype=np.float32)
    sw = rng.standard_normal(S_TOTAL, dtype=np.float32)
    out = kernel(x, sw)
    print("kernel output shape:", out.shape, out.dtype)
